# revision 3
# baseline (speedup 1.0000x reference)
"""Bass/Tile TRN2 kernel for nn_MultiHeadAttention_9277129359942.

B=2, T=S=2048, D=1024, H=16 heads, head_dim=64, fp32 I/O.

Sharding (8 cores): data-parallel over batch (2) x tensor-parallel over
head groups (4 heads / core, 256 out dims).  Each core computes the
attention for its 4 heads and a partial output projection; the host sums
the 4 partials per batch (row-parallel Wo) plus per-core bo/4 terms.

v2 design notes (vs the v1 baseline, 430us profiled):
  - The attention inner loop was scalar-engine bound: 2 exact exp()
    activations per s-iteration (2294ns) vs 1280ns of PE work, which
    starved the PE and kept the HAM clock gate at K=4/8 (1.2 GHz) for
    250us.  v2 splits the exp work: 1 of 4 score chunks per iteration
    uses the exact ACT exp (720ns), the other 3 use a one-instruction
    DVE fast-exp (Schraudolph: round(x*128*log2e*0.125 + magic) ->
    int16, bitcast as bf16).  End-to-end rel err predicted 1.63e-2
    (gate 2e-2), verified by numpy emulation against the reference.
  - Software pipelining: head A's ctx matmuls are deferred by one
    s-iteration so the ACT exp latency is hidden; head B's ctx follows
    its DVE fast-exp in the same iteration.  PSUM: scA 2 + scB 2 +
    ctxA 2 + ctxB 2 = 8 banks exactly.
  - x DMAs are issued k-tile round-robin (q,k first, v later) so the
    first projection matmuls start ~2us in; a short PE warmup burst
    during the DMA head un-throttles the HAM clock early; q/k psum
    eviction+bias moved to the (otherwise idle) scalar engine.
  - Out-projection runs at the tail in 512-wide chunks; the last
    block's softmax denominators are broadcast with a K=1 PE matmul
    instead of the log2 SBUF DMA chain so the final normalize does not
    stall the tail.
"""

import os
import sys

import numpy as np

for _p in ("/opt/trn_rl_repo",):
    if os.path.isdir(_p) and _p not in sys.path:
        sys.path.append(_p)

import ml_dtypes

import concourse.bass as bass
import concourse.mybir as mybir
import concourse.tile as tile
from concourse import bacc
from concourse.bass_utils import run_bass_kernel_spmd

F32 = mybir.dt.float32
BF16 = mybir.dt.bfloat16
I16 = mybir.dt.int16
AF = mybir.ActivationFunctionType
ALU = mybir.AluOpType
BF16_NP = ml_dtypes.bfloat16

D = 1024          # model dim
T = 2048          # query length
S = 2048          # key length
P = 128           # partitions
KT = D // P       # 8 contraction tiles
TT = T // P       # 16 row tiles
ST = S // P       # 16 key tiles
HL = 4            # local heads per core
HD = 64           # head dim
OUTL = HL * HD    # 256 local out dims
VW = HD + 1       # v_aug width per head (ones column appended)
N_CORES = 8

# fast-exp constants: exp(x*0.125) ~= bf16(bitcast(int16(x*EA + EC)))
EA = float(0.125 * 128.0 / np.log(2.0))
EC = float(127 * 128 - 7.5)


def build_program():
    """Build + compile the SPMD program (same on all 8 cores)."""
    nc = bacc.Bacc(
        "TRN2", target_bir_lowering=False, debug=False, enable_asserts=True,
        num_devices=N_CORES,
    )

    xq_d = nc.dram_tensor("xq", [D, T], BF16, kind="ExternalInput")
    xk_d = nc.dram_tensor("xk", [D, S], BF16, kind="ExternalInput")
    xv_d = nc.dram_tensor("xv", [D, S], BF16, kind="ExternalInput")
    wq_d = nc.dram_tensor("wq", [D, OUTL], BF16, kind="ExternalInput")
    wk_d = nc.dram_tensor("wk", [D, OUTL], BF16, kind="ExternalInput")
    wv_d = nc.dram_tensor("wv", [D, OUTL], BF16, kind="ExternalInput")
    wo_d = nc.dram_tensor("wo", [OUTL, D], BF16, kind="ExternalInput")
    bq_d = nc.dram_tensor("bq", [OUTL, 1], F32, kind="ExternalInput")
    bk_d = nc.dram_tensor("bk", [OUTL, 1], F32, kind="ExternalInput")
    bv_d = nc.dram_tensor("bv_rep", [P, OUTL], F32, kind="ExternalInput")
    bo_d = nc.dram_tensor("bo4_rep", [P, D], F32, kind="ExternalInput")
    out_d = nc.dram_tensor("out", [T, D], F32, kind="ExternalOutput")
    wsink_d = nc.dram_tensor("warm_sink", [1, 8], F32, kind="ExternalOutput")

    with tile.TileContext(nc) as tc:
        _build(nc, tc, xq_d, xk_d, xv_d, wq_d, wk_d, wv_d, wo_d,
               bq_d, bk_d, bv_d, bo_d, out_d, wsink_d)
    nc.compile()
    return nc


def _build(nc, tc, xq_d, xk_d, xv_d, wq_d, wk_d, wv_d, wo_d,
           bq_d, bk_d, bv_d, bo_d, out_d, wsink_d):
    from contextlib import ExitStack

    stack = ExitStack()
    with stack:
        consts = stack.enter_context(tc.tile_pool(name="consts", bufs=1))
        wpool = stack.enter_context(tc.tile_pool(name="wpool", bufs=1))
        acts = stack.enter_context(tc.tile_pool(name="acts", bufs=1))
        npool = stack.enter_context(tc.tile_pool(name="npool", bufs=2))
        nrmpool = stack.enter_context(tc.tile_pool(name="nrmpool", bufs=1))

        # ---- constants + weights (issue order = DMA priority) ----------
        bq_sb = consts.tile([P, 2], F32, name="bq", tag="bq")
        bk_sb = consts.tile([P, 2], F32, name="bk", tag="bk")
        bv_sb = consts.tile([P, OUTL], F32, name="bv", tag="bv")
        bo_sb = consts.tile([P, D], F32, name="bo", tag="bo")
        ones_sb = consts.tile([1, HD], F32, name="ones", tag="ones")
        wsnk = consts.tile([1, 8], F32, name="wsnk", tag="wsnk")

        wq_sb = [wpool.tile([P, OUTL], BF16, name=f"wq{k}", tag=f"wq{k}")
                 for k in range(KT)]
        wk_sb = [wpool.tile([P, OUTL], BF16, name=f"wk{k}", tag=f"wk{k}")
                 for k in range(KT)]
        wv_sb = [wpool.tile([P, OUTL], BF16, name=f"wv{k}", tag=f"wv{k}")
                 for k in range(KT)]
        wo_sb = [wpool.tile([P, D], BF16, name=f"wo{k}", tag=f"wo{k}")
                 for k in range(2)]

        # persistent activation tensors
        qT = [acts.tile([P, T], BF16, name=f"qT{m}", tag=f"qT{m}")
              for m in range(2)]
        kT = [acts.tile([P, S], BF16, name=f"kT{m}", tag=f"kT{m}")
              for m in range(2)]
        v_aug = acts.tile([P, ST * HL * VW], BF16, name="vaug", tag="vaug")
        ctxT = [[acts.tile([P, 1024], BF16, name=f"ctxT{g}{th}",
                           tag=f"ctxT{g}{th}") for th in range(2)]
                for g in range(2)]

        xpool = stack.enter_context(tc.tile_pool(name="xpool", bufs=1))
        xq_sb = [xpool.tile([P, T], BF16, name=f"xq{k}", tag=f"xq{k}")
                 for k in range(KT)]
        xk_sb = [xpool.tile([P, S], BF16, name=f"xk{k}", tag=f"xk{k}")
                 for k in range(KT)]
        xv_sb = [xpool.tile([P, S], BF16, name=f"xv{k}", tag=f"xv{k}")
                 for k in range(KT)]

        def load4(sb, d_, k):
            for q in range(4):
                nc.sync.dma_start(sb[k][q * 32:(q + 1) * 32, :],
                                  d_[k * P + q * 32:k * P + (q + 1) * 32, :])

        # DMA priority: wq0/wk0 + biases, then xq/xk k-round-robin (the
        # q/k projections chase these), then bv/bo/wv, then xv, then wo.
        nc.sync.dma_start(wq_sb[0][:], wq_d[0:P, :])
        nc.sync.dma_start(wk_sb[0][:], wk_d[0:P, :])
        nc.sync.dma_start(bq_sb[:], bq_d.rearrange("(m p) o -> p (m o)", p=P))
        nc.sync.dma_start(bk_sb[:], bk_d.rearrange("(m p) o -> p (m o)", p=P))
        load4(xq_sb, xq_d, 0)
        load4(xk_sb, xk_d, 0)
        for k in range(1, KT):
            nc.sync.dma_start(wq_sb[k][:], wq_d[k * P:(k + 1) * P, :])
            nc.sync.dma_start(wk_sb[k][:], wk_d[k * P:(k + 1) * P, :])
            load4(xq_sb, xq_d, k)
            load4(xk_sb, xk_d, k)
        nc.sync.dma_start(bv_sb[:], bv_d[:, :])
        nc.sync.dma_start(bo_sb[:], bo_d[:, :])
        for k in range(KT):
            nc.sync.dma_start(wv_sb[k][:], wv_d[k * P:(k + 1) * P, :])
        for k in range(KT):
            load4(xv_sb, xv_d, k)
        for k in range(2):
            nc.sync.dma_start(wo_sb[k][:], wo_d[k * P:(k + 1) * P, :])

        nc.vector.memset(v_aug[:], 1.0)  # ones columns survive the v writes
        nc.vector.memset(ones_sb[:], 1.0)

        # ACT table preload: tiny exp as the first scalar-engine op so the
        # ~2.7us exp table DMA happens during the x loads.
        nc.scalar.activation(wsnk[0:1, 0:2], bq_sb[0:1, 0:2], AF.Exp)

        # ---- q/k projections (psum double-buffered, ACT eviction) ------
        with tc.tile_pool(name="qkpsum", bufs=2, space="PSUM") as qkpsum:
            # PE warmup burst: un-throttle the HAM clock gate during the
            # DMA head; overwritten by the first real projection psum.
            warm = qkpsum.tile([P, T], F32, name="pqk", tag="pqk")
            for w in range(16):
                nc.tensor.matmul(warm[:, 0:OUTL], wq_sb[0][:, 0:P],
                                 wq_sb[0][:], start=(w == 0), stop=(w == 15))
            nc.vector.tensor_copy(wsnk[0:1, 2:4], warm[0:1, 0:2])
            nc.sync.dma_start(wsink_d[:, :], wsnk[:])

            for m in range(2):
                for w_sb, x_sb, b_sb, o_sb in (
                        (wq_sb, xq_sb, bq_sb, qT),
                        (wk_sb, xk_sb, bk_sb, kT)):
                    ps = qkpsum.tile([P, T], F32, name="pqk", tag="pqk")
                    for k in range(KT):
                        for c in range(4):
                            cs = slice(c * 512, (c + 1) * 512)
                            nc.tensor.matmul(
                                ps[:, cs], w_sb[k][:, m * P:(m + 1) * P],
                                x_sb[k][:, cs],
                                start=(k == 0), stop=(k == KT - 1))
                    nc.scalar.activation(o_sb[m][:], ps[:], AF.Identity,
                                         bias=b_sb[:, m:m + 1])

        # ---- v projection (natural [S, 256] layout into v_aug) ---------
        bv3 = bv_sb[:].rearrange("p (h x) -> p h x", x=HD)
        with tc.tile_pool(name="vpsum", bufs=2, space="PSUM") as vpsum:
            for s in range(ST):
                ps = vpsum.tile([P, OUTL], F32, name="pv", tag="pv")
                for k in range(KT):
                    nc.tensor.matmul(
                        ps[:], xv_sb[k][:, s * P:(s + 1) * P], wv_sb[k][:],
                        start=(k == 0), stop=(k == KT - 1))
                dst = v_aug[:, s * HL * VW:(s + 1) * HL * VW]
                dst = dst.rearrange("p (h x) -> p h x", x=VW)[:, :, 0:HD]
                nc.vector.tensor_tensor(
                    out=dst, in0=ps[:].rearrange("p (h x) -> p h x", x=HD),
                    in1=bv3, op=ALU.add)

        # ---- attention: 4 blocks of (head pair p, t-half th) -----------
        # per s-iteration: scores for heads A/B run concurrently in
        # disjoint PE row groups; head A chunk0 exp on ACT (exact), the
        # other 3 chunks on DVE fast-exp; ctxA deferred one iteration.
        last_norm = None
        with tc.tile_pool(name="scpsum", bufs=1, space="PSUM") as scpsum, \
             tc.tile_pool(name="ctxpsum", bufs=1, space="PSUM") as ctxpsum, \
             tc.tile_pool(name="epool", bufs=2) as epool:

            for p in range(2):          # head pair (local heads 2p, 2p+1)
                for th in range(2):     # t halves of 1024
                    t0 = th * 1024
                    ctxA = ctxpsum.tile([VW, 1024], F32, name="ctxA",
                                        tag="ctxA")
                    ctxB = ctxpsum.tile([VW, 1024], F32, name="ctxB",
                                        tag="ctxB")
                    hA = 2 * p
                    hB = 2 * p + 1

                    def vslice(h, s):
                        return slice(s * HL * VW + h * VW,
                                     s * HL * VW + (h + 1) * VW)

                    def emit_ctxA(sp, exA, eiA):
                        ebA = eiA[:].bitcast(BF16)
                        for c, mov in ((0, exA[:]), (1, ebA)):
                            nc.tensor.matmul(
                                ctxA[:, c * 512:(c + 1) * 512],
                                v_aug[:, vslice(hA, sp)], mov,
                                start=(sp == 0), stop=(sp == ST - 1))

                    prev = None
                    for s in range(ST):
                        ss = slice(s * P, (s + 1) * P)
                        scA = scpsum.tile([P, 1024], F32, name="scA",
                                          tag="scA")
                        scB = scpsum.tile([P, 1024], F32, name="scB",
                                          tag="scB")
                        for c in range(2):
                            cs = slice(c * 512, (c + 1) * 512)
                            ts_ = slice(t0 + c * 512, t0 + (c + 1) * 512)
                            nc.tensor.matmul(scA[:, cs], kT[p][0:HD, ss],
                                             qT[p][0:HD, ts_],
                                             start=True, stop=True)
                            nc.tensor.matmul(scB[:, cs], kT[p][HD:P, ss],
                                             qT[p][HD:P, ts_],
                                             start=True, stop=True)
                        exA = epool.tile([P, 512], BF16, name="exA",
                                         tag="exA")
                        eiA = epool.tile([P, 512], I16, name="eiA",
                                         tag="eiA")
                        eiB = epool.tile([P, 1024], I16, name="eiB",
                                         tag="eiB")
                        nc.scalar.activation(exA[:], scA[:, 0:512], AF.Exp,
                                             scale=0.125)
                        nc.vector.tensor_scalar(eiB[:, 0:512], scB[:, 0:512],
                                                EA, EC, op0=ALU.mult,
                                                op1=ALU.add)
                        nc.vector.tensor_scalar(eiB[:, 512:1024],
                                                scB[:, 512:1024],
                                                EA, EC, op0=ALU.mult,
                                                op1=ALU.add)
                        nc.vector.tensor_scalar(eiA[:], scA[:, 512:1024],
                                                EA, EC, op0=ALU.mult,
                                                op1=ALU.add)
                        if prev is not None:
                            emit_ctxA(*prev)
                        ebB = eiB[:].bitcast(BF16)
                        for c in range(2):
                            cs = slice(c * 512, (c + 1) * 512)
                            nc.tensor.matmul(ctxB[:, cs],
                                             v_aug[:, vslice(hB, s)],
                                             ebB[:, cs],
                                             start=(s == 0),
                                             stop=(s == ST - 1))
                        prev = (s, exA, eiA)
                    emit_ctxA(*prev)

                    # evict ctx+denoms, start reciprocals; the normalize
                    # itself is cheap DVE work once the 1/denom row is
                    # broadcast across partitions.
                    stgA = npool.tile([VW, 1024], F32, name="stgA",
                                      tag="stgA")
                    stgB = npool.tile([VW, 1024], F32, name="stgB",
                                      tag="stgB")
                    nc.vector.tensor_copy(stgA[:], ctxA[:])
                    nc.vector.tensor_copy(stgB[:], ctxB[:])

                    if (p, th) != (1, 1):
                        # log2 SBUF DMA broadcast chain on the scalar
                        # engine's DMA queue (off the critical path)
                        for i, stg in ((0, stgA), (1, stgB)):
                            rb = nrmpool.tile([HD, 1024], F32,
                                              name=f"rb{i}", tag=f"rb{i}")
                            nc.vector.reciprocal(rb[0:1, :],
                                                 stg[HD:HD + 1, :])
                            w = 1
                            while w < HD:
                                nc.scalar.dma_start(rb[w:2 * w, :],
                                                    rb[0:w, :])
                                w *= 2
                            if i == 0:
                                nc.vector.tensor_tensor(
                                    out=ctxT[p][th][0:HD, :],
                                    in0=stg[0:HD, :], in1=rb[:],
                                    op=ALU.mult)
                            else:
                                ostg = nrmpool.tile([HD, 1024], BF16,
                                                    name="ostg", tag="ostg")
                                nc.vector.tensor_tensor(
                                    out=ostg[:], in0=stg[0:HD, :],
                                    in1=rb[:], op=ALU.mult)
                                nc.scalar.dma_start(
                                    ctxT[p][th][HD:P, :], ostg[:])
                    else:
                        # last block: defer; normalize via PE broadcast
                        # overlapped with the first out-projection tiles
                        rr = []
                        for i, stg in ((0, stgA), (1, stgB)):
                            r0 = nrmpool.tile([1, 1024], F32,
                                              name=f"r0{i}", tag=f"r0{i}")
                            nc.vector.reciprocal(r0[:], stg[HD:HD + 1, :])
                            rr.append(r0)
                        last_norm = (stgA, stgB, rr)

        # ---- output projection + last-block normalize ------------------
        with tc.tile_pool(name="tpsum", bufs=1, space="PSUM") as tpsum, \
             tc.tile_pool(name="popsum", bufs=2, space="PSUM") as popsum, \
             tc.tile_pool(name="opool", bufs=3) as opool:

            def emit_outproj(trange):
                for t in trange:
                    th_, tt_ = divmod(t, TT // 2)
                    ts_ = slice(tt_ * P, (tt_ + 1) * P)
                    for n in range(2):
                        ns = slice(n * 512, (n + 1) * 512)
                        po = popsum.tile([P, 512], F32, name="po", tag="po")
                        for g in range(2):
                            nc.tensor.matmul(po[:], ctxT[g][th_][:, ts_],
                                             wo_sb[g][:, ns],
                                             start=(g == 0), stop=(g == 1))
                        ost = opool.tile([P, 512], F32, name="ost",
                                         tag="ost")
                        nc.vector.tensor_tensor(out=ost[:], in0=po[:],
                                                in1=bo_sb[:, ns],
                                                op=ALU.add)
                        for q in range(4):
                            nc.sync.dma_start(
                                out_d[t * P + q * 32:t * P + (q + 1) * 32,
                                      ns],
                                ost[q * 32:(q + 1) * 32, :])

            emit_outproj(range(0, 4))

            # last-block normalize: K=1 matmul broadcasts 1/denom to 64
            # partitions (fp32, exact); runs between out-proj tiles.
            stgA, stgB, rr = last_norm
            rbps = [tpsum.tile([HD, 1024], F32, name=f"rbps{i}",
                               tag=f"rbps{i}") for i in range(2)]
            for i in range(2):
                for c in range(2):
                    cs = slice(c * 512, (c + 1) * 512)
                    nc.tensor.matmul(rbps[i][:, cs], ones_sb[:],
                                     rr[i][0:1, cs], start=True, stop=True)
            nc.vector.tensor_tensor(out=ctxT[1][1][0:HD, :],
                                    in0=stgA[0:HD, :], in1=rbps[0][:],
                                    op=ALU.mult)
            ostg = nrmpool.tile([HD, 1024], BF16, name="ostgL", tag="ostg")
            nc.vector.tensor_tensor(out=ostg[:], in0=stgB[0:HD, :],
                                    in1=rbps[1][:], op=ALU.mult)
            nc.scalar.dma_start(ctxT[1][1][HD:P, :], ostg[:])

            emit_outproj(range(4, TT))


def make_in_maps(query, key, value, Wq, bq, Wk, bk, Wv, bv, Wo, bo):
    """Shard the full inputs into the 8 per-core input dicts."""
    query, key, value, Wq, bq, Wk, bk, Wv, bv, Wo, bo = [
        np.asarray(a, dtype=np.float32)
        for a in (query, key, value, Wq, bq, Wk, bk, Wv, bv, Wo, bo)]

    def bf(a):
        return np.ascontiguousarray(a).astype(BF16_NP)

    in_maps = []
    for c in range(N_CORES):
        b, g = divmod(c, 4)
        sl = slice(g * OUTL, (g + 1) * OUTL)
        in_maps.append({
            "xq": bf(query[b].T),
            "xk": bf(key[b].T),
            "xv": bf(value[b].T),
            "wq": bf(Wq[sl, :].T),
            "wk": bf(Wk[sl, :].T),
            "wv": bf(Wv[sl, :].T),
            "wo": bf(Wo[:, sl].T),
            "bq": np.ascontiguousarray(bq[sl].reshape(OUTL, 1)),
            "bk": np.ascontiguousarray(bk[sl].reshape(OUTL, 1)),
            "bv_rep": np.ascontiguousarray(
                np.broadcast_to(bv[sl], (P, OUTL))),
            "bo4_rep": np.ascontiguousarray(
                np.broadcast_to(bo * 0.25, (P, D))),
        })
    return in_maps


_NC_CACHE = None


def _get_nc():
    global _NC_CACHE
    if _NC_CACHE is None:
        _NC_CACHE = build_program()
    return _NC_CACHE


def kernel(query, key, value, Wq, bq, Wk, bk, Wv, bv, Wo, bo):
    nc = _get_nc()
    in_maps = make_in_maps(query, key, value, Wq, bq, Wk, bk, Wv, bv, Wo, bo)
    res = run_bass_kernel_spmd(nc, in_maps, list(range(N_CORES))).results
    out = np.empty((2, T, D), dtype=np.float32)
    for b in range(2):
        acc = res[4 * b]["out"].astype(np.float32, copy=True)
        for g in range(1, 4):
            acc += res[4 * b + g]["out"]
        out[b] = acc
    return out
